# revision 6
# baseline (speedup 1.0000x reference)
"""Bass/Trainium2 kernel for batched dot-product attention.

Problem: q,k,v [B=4, S=4096, D=1024]; projections to dk=dv=128; softmax
attention per batch element.  Sharded over 8 NeuronCores as (batch,
KEY-half): core c handles batch c//2, keys (c%2)*2048 ... +2048, and ALL
4096 queries.  Each core emits unnormalized partial outputs o^T [dv, q]
and partial softmax denominators s [1, q]; the host combines
(o0+o1)/(s0+s1) — mathematically identical to full softmax (no max
subtraction is used anywhere, matching the reference).

Algebraic simplifications (exact):
  - bk dropped: scores' qp.bk term is constant per query row -> softmax
    invariant.
  - bv dropped on-chip: attn rows sum to 1 after normalization, so bv is
    added on the host after the combine.
  - scale 1/sqrt(dk) folded into wq/bq on the host.

On-chip layouts keep the contraction dim on SBUF partitions:
  qT/kT/vT   [d_model, seq]   (host pre-transposed, bf16)
  kpT/qpT    [dk, seq]        (projection output, bf16)
  vp         [keys, dv]       (via PE transpose, bf16)
  S^T tiles  [keys, q]        (scores transposed, PSUM)
  out^T      [dv, q]          (partial output, f32)

All 4 query-block pairs run DEFERRED: window p streams S matmuls + exp
for pair p while the PE also drains pair p-1's AV matmul burst and any
remaining projections — the PE queue always has non-ScalarE-dependent
work so exp latency never stalls it.  Softmax denominators accumulate in
bf16 on VectorE (4x DVE mode), reduced by a ones-vector matmul (bf16
moving operand = full PE rate).  Warm-up matmuls at t~5.5us (on the
freshly landed wq tile) ramp the PE HAM clock gate to 2.4GHz before the
first projection and soak up the initial input-DMA latency.
"""

import math

import numpy as np
import ml_dtypes

import concourse.bass as bass
import concourse.tile as tile
from concourse import bacc, mybir
from concourse.bass_utils import run_bass_kernel_spmd

B, S, DM, DK, DV = 4, 4096, 1024, 128, 128
N_CORES = 8
SK = S // 2          # keys per core
NQB = S // 512       # query blocks of 512 (8)
NKB = SK // 512      # key blocks of 512 per core (4)
NKC = SK // 128      # key chunks of 128 per core (16)
NMC = DM // 128      # d_model chunks (8)
NPAIR = NQB // 2     # query-block pairs (4)

BF16 = mybir.dt.bfloat16
F32 = mybir.dt.float32
NP_BF16 = ml_dtypes.bfloat16

WARMUP_MM = 12       # junk matmuls to ramp HAM during initial DMA wait
EXP_STAG = 1         # chunks the exp lags the S matmuls
AV_STAG = 2          # chunks the inline AV lags the exp (last pair)

Copy = mybir.ActivationFunctionType.Copy
Exp = mybir.ActivationFunctionType.Exp


def _emit(tc: tile.TileContext, aps: dict):
    nc = tc.nc
    qT, kT, vT = aps["qT"], aps["kT"], aps["vT"]
    outT = aps["outT"]

    with tc.tile_pool(name="persist", bufs=1) as persist:
        # --- warmup operand: zeros via memset, no DMA dependency ---
        wz = persist.tile([128, 512], BF16, tag="wz")
        nc.gpsimd.memset(wz[:], 0.0)

        w_sb = {n: persist.tile([128, NMC, 128], BF16, tag=f"w_{n}", name=f"w_{n}")
                for n in ("wq", "wk", "wv")}
        bias_sb = persist.tile([128, 1], F32, tag="bias")
        ones_sb = persist.tile([128, 1], BF16, tag="ones")
        ident_sb = persist.tile([128, 128], BF16, tag="ident")

        # --- persistent activations ---
        kpT_blk = [persist.tile([128, 512], BF16, tag=f"kpT{i}", name=f"kpT{i}")
                   for i in range(NKB)]
        qpT_t = [persist.tile([128, 512], BF16, tag=f"qpT{i}", name=f"qpT{i}")
                 for i in range(NQB)]
        vp_pair = [persist.tile([128, 256], BF16, tag=f"vpp{i}", name=f"vpp{i}")
                   for i in range(NKC // 2)]
        sums_sb = persist.tile([1, S], F32, tag="sums", name="sums_sb")

        with (
            tc.tile_pool(name="op", bufs=2, space="PSUM") as op,
            tc.tile_pool(name="pp", bufs=2, space="PSUM") as pp,
            tc.tile_pool(name="sp", bufs=2, space="PSUM") as sp,
            tc.tile_pool(name="xs", bufs=2) as xs,
            tc.tile_pool(name="ep", bufs=3) as ep,
            tc.tile_pool(name="e1p", bufs=1) as e1p,
            tc.tile_pool(name="accp", bufs=2) as accp,
            tc.tile_pool(name="miscp", bufs=2) as miscp,
        ):
            # ---- warmup matmuls: ramp HAM while inputs stream in ----
            for i in range(WARMUP_MM):
                wt = pp.tile([128, 512], F32, tag="pp", name=f"warm{i}")
                nc.tensor.matmul(wt[:], lhsT=wz[:, 0:128], rhs=wz[:],
                                 start=True, stop=True)

            # ---- input fetch helpers ----
            kxs, vxs, qxs = {}, {}, {}

            def fetch_kx(kb, eng):
                kx = xs.tile([128, NMC, 512], BF16, tag="kx", name=f"kx{kb}",
                             bufs=2)
                eng.dma_start(kx[:], kT[kb])
                kxs[kb] = kx

            def fetch_vx(kb, eng):
                vx = xs.tile([128, NMC, 512], BF16, tag="vx", name=f"vx{kb}",
                             bufs=2)
                eng.dma_start(vx[:], vT[kb])
                vxs[kb] = vx

            def fetch_q(qb, eng):
                qx = xs.tile([128, NMC, 512], BF16, tag="qx", name=f"qx{qb}",
                             bufs=3)
                eng.dma_start(qx[:], qT[qb])
                qxs[qb] = qx

            # ---- DMA issue: 4 queues in parallel, priority order ----
            fetch_kx(0, nc.scalar)
            nc.scalar.dma_start(w_sb["wk"][:], aps["wk"][:])
            nc.scalar.dma_start(w_sb["wq"][:], aps["wq"][:])

            fetch_q(0, nc.sync)
            fetch_q(1, nc.sync)
            nc.sync.dma_start(bias_sb[:], aps["bq"][:])
            nc.sync.dma_start(w_sb["wv"][:], aps["wv"][:])
            fetch_kx(1, nc.sync)
            fetch_kx(2, nc.sync)
            fetch_kx(3, nc.sync)
            fetch_vx(0, nc.sync)
            fetch_q(2, nc.sync)
            fetch_q(3, nc.sync)
            fetch_vx(1, nc.sync)
            fetch_q(4, nc.sync)
            fetch_q(5, nc.sync)

            nc.gpsimd.dma_start(ident_sb[:], aps["ident"][:])
            nc.gpsimd.dma_start(ones_sb[:], aps["ones"][:])
            fetch_vx(2, nc.gpsimd)
            fetch_vx(3, nc.gpsimd)
            fetch_q(6, nc.gpsimd)
            fetch_q(7, nc.gpsimd)

            # preload the Exp activation table off the critical path
            et = miscp.tile([1, 1], F32, tag="misc", name="exp_preload")
            nc.scalar.activation(et[:], bias_sb[0:1, 0:1], Exp)

            # ---- projections ----
            def proj_k(kb):
                kx = kxs.pop(kb)
                psk = pp.tile([128, 512], F32, tag="pp", name=f"psk{kb}")
                for c in range(NMC):
                    nc.tensor.matmul(
                        psk[:], lhsT=w_sb["wk"][:, c, :], rhs=kx[:, c, :],
                        start=(c == 0), stop=(c == NMC - 1),
                    )
                nc.vector.tensor_copy(kpT_blk[kb][:], psk[:])

            def proj_q(qb):
                qx = qxs.pop(qb)
                psq = pp.tile([128, 512], F32, tag="pp", name=f"psq{qb}")
                for c in range(NMC):
                    nc.tensor.matmul(
                        psq[:], lhsT=w_sb["wq"][:, c, :], rhs=qx[:, c, :],
                        start=(c == 0), stop=(c == NMC - 1),
                    )
                nc.vector.tensor_scalar_add(qpT_t[qb][:], psq[:], bias_sb[:, 0:1])

            def proj_v(kb):
                vx = vxs.pop(kb)
                psv = pp.tile([128, 512], F32, tag="pp", name=f"psv{kb}")
                for c in range(NMC):
                    nc.tensor.matmul(
                        psv[:], lhsT=w_sb["wv"][:, c, :], rhs=vx[:, c, :],
                        start=(c == 0), stop=(c == NMC - 1),
                    )
                vpt = xs.tile([128, 512], BF16, tag="vpt", name=f"vpt{kb}")
                nc.vector.tensor_copy(vpt[:], psv[:])
                for j in range(2):
                    tp = pp.tile([128, 256], BF16, tag="pp", name=f"tp{kb}_{j}")
                    for i in range(2):
                        nc.tensor.transpose(
                            tp[:, i * 128:(i + 1) * 128],
                            vpt[:, (2 * j + i) * 128:(2 * j + i + 1) * 128],
                            ident_sb[:],
                        )
                    nc.vector.tensor_copy(vp_pair[2 * kb + j][:], tp[:])

            # ---- attention pair machinery ----
            # pairs 0..2: deferred AV (burst interleaved into next window);
            # pair 3: inline AV into the pp-pool banks (projections done).
            def pair_begin(p, defer):
                return dict(
                    p=p, qs=(2 * p, 2 * p + 1), defer=defer,
                    o=None,
                    acc=accp.tile([128, 1024], BF16, tag="acc", name=f"acc{p}"),
                    pend=[], evs=[], elast=None,
                )

            def av_emit(st, kc, e):
                vps = vp_pair[kc // 2][:, (kc % 2) * 128:(kc % 2 + 1) * 128]
                for h in range(2):
                    nc.tensor.matmul(
                        st["o"][h][:], lhsT=vps, rhs=e[:, h * 512:(h + 1) * 512],
                        start=(kc == 0), stop=(kc == NKC - 1),
                    )

            def pair_drain(st):
                kc, s = st["pend"].pop(0)
                if st["defer"]:
                    e = e1p.tile([128, 1024], BF16, tag=f"e{st['p'] % 2}_{kc}",
                                 name=f"e{st['p']}_{kc}")
                else:
                    e = ep.tile([128, 1024], BF16, tag="e", name=f"e{st['p']}_{kc}")
                nc.scalar.activation(e[:], s[:], Exp)
                if kc % 2 == 0:
                    st["elast"] = e
                else:
                    tmp = ep.tile([128, 1024], BF16, tag="tmp",
                                  name=f"t{st['p']}_{kc}")
                    nc.vector.tensor_add(tmp[:], st["elast"][:], e[:])
                    if kc == 1:
                        nc.vector.tensor_copy(st["acc"][:], tmp[:])
                    else:
                        nc.vector.tensor_add(st["acc"][:], st["acc"][:], tmp[:])
                if st["defer"]:
                    st["evs"].append((kc, e))
                else:
                    st["evs"].append((kc, e))
                    while len(st["evs"]) > AV_STAG:
                        kc2, e2 = st["evs"].pop(0)
                        av_emit(st, kc2, e2)

            def pair_chunk(st, kc):
                s = sp.tile([128, 1024], F32, tag="sp", name=f"s{st['p']}_{kc}")
                kslice = kpT_blk[kc // 4][:, (kc % 4) * 128:(kc % 4 + 1) * 128]
                for h in range(2):
                    nc.tensor.matmul(
                        s[:, h * 512:(h + 1) * 512], lhsT=kslice,
                        rhs=qpT_t[st["qs"][h]][:], start=True, stop=True,
                    )
                st["pend"].append((kc, s))
                if len(st["pend"]) > EXP_STAG:
                    pair_drain(st)

            def pair_flush(st):
                while st["pend"]:
                    pair_drain(st)
                if not st["defer"]:
                    while st["evs"]:
                        kc2, e2 = st["evs"].pop(0)
                        av_emit(st, kc2, e2)

            def burst_begin(st, pool):
                st["o"] = [pool.tile([128, 512], F32,
                                     tag="op" if pool is op else "pp",
                                     name=f"o{q}") for q in st["qs"]]

            def burst_chunk(st, i):
                kc, e = st["evs"][i]
                av_emit(st, kc, e)

            def pair_sums(st):
                for h, q in enumerate(st["qs"]):
                    ps_sum = pp.tile([1, 512], F32, tag="pp", name=f"pssum{q}")
                    nc.tensor.matmul(
                        ps_sum[:], lhsT=ones_sb[:],
                        rhs=st["acc"][:, h * 512:(h + 1) * 512],
                        start=True, stop=True,
                    )
                    nc.vector.tensor_copy(sums_sb[:, q * 512:(q + 1) * 512],
                                          ps_sum[:])

            def pair_out(st):
                for h, q in enumerate(st["qs"]):
                    outsb = miscp.tile([128, 512], F32, tag="out", name=f"out{q}")
                    nc.vector.tensor_copy(outsb[:], st["o"][h][:])
                    nc.sync.dma_start(outT[:, q * 512:(q + 1) * 512], outsb[:])

            # ---- startup projections ----
            proj_k(0)
            proj_q(0)
            proj_q(1)

            sts = [pair_begin(p, defer=(p < NPAIR - 1)) for p in range(NPAIR)]

            # per-window extra PE work, emitted after given chunk index
            extras = {
                0: {2: [lambda: proj_k(1)],
                    5: [lambda: proj_v(0)],
                    6: [lambda: proj_k(2)],
                    8: [lambda: proj_v(1)],
                    10: [lambda: proj_k(3)],
                    12: [lambda: proj_q(2)],
                    14: [lambda: proj_q(3)]},
                1: {4: [lambda: proj_v(2)],
                    8: [lambda: proj_v(3)],
                    11: [lambda: proj_q(4)],
                    14: [lambda: proj_q(5)]},
                2: {4: [lambda: proj_q(6)],
                    8: [lambda: proj_q(7)]},
                3: {},
            }

            for p in range(NPAIR):
                st = sts[p]
                prev = sts[p - 1] if p > 0 else None
                if prev is not None:
                    burst_begin(prev, op)
                if not st["defer"]:
                    burst_begin(st, pp)
                for kc in range(NKC):
                    pair_chunk(st, kc)
                    if prev is not None:
                        burst_chunk(prev, kc)
                    for fn in extras[p].get(kc, ()):
                        fn()
                pair_flush(st)
                if prev is not None:
                    pair_sums(prev)
                    pair_out(prev)

            # tail: last pair was inline — only sums + output remain
            last = sts[-1]
            pair_sums(last)
            pair_out(last)
            nc.sync.dma_start(aps["sums"][:], sums_sb[:])


_CACHE = {}


def _build():
    if "nc" in _CACHE:
        return _CACHE["nc"]
    nc = bacc.Bacc("TRN2", debug=False, num_devices=N_CORES)
    aps = {
        "qT": nc.dram_tensor("qT", [NQB, 128, NMC, 512], BF16,
                             kind="ExternalInput").ap(),
        "kT": nc.dram_tensor("kT", [NKB, 128, NMC, 512], BF16,
                             kind="ExternalInput").ap(),
        "vT": nc.dram_tensor("vT", [NKB, 128, NMC, 512], BF16,
                             kind="ExternalInput").ap(),
        "wq": nc.dram_tensor("wq", [128, NMC, DK], BF16, kind="ExternalInput").ap(),
        "wk": nc.dram_tensor("wk", [128, NMC, DK], BF16, kind="ExternalInput").ap(),
        "wv": nc.dram_tensor("wv", [128, NMC, DV], BF16, kind="ExternalInput").ap(),
        "bq": nc.dram_tensor("bq", [128, 1], F32, kind="ExternalInput").ap(),
        "ones": nc.dram_tensor("ones", [128, 1], BF16, kind="ExternalInput").ap(),
        "ident": nc.dram_tensor("ident", [128, 128], BF16,
                                kind="ExternalInput").ap(),
        "outT": nc.dram_tensor("outT", [DV, S], F32, kind="ExternalOutput").ap(),
        "sums": nc.dram_tensor("sums", [1, S], F32, kind="ExternalOutput").ap(),
    }
    with tile.TileContext(nc) as tc:
        _emit(tc, aps)
    nc.compile()
    _CACHE["nc"] = nc
    return nc


def _pack_w(w):
    # [DM, d] -> [128, NMC, d]  (chunk-major weight layout)
    return np.ascontiguousarray(np.asarray(w).reshape(NMC, 128, -1).transpose(1, 0, 2))


def _pack_x(xT, nblk):
    # [DM, n] -> [nblk, 128, NMC, 512]  (contiguous per-stripe layout)
    return np.ascontiguousarray(
        xT.reshape(NMC, 128, nblk, 512).transpose(2, 1, 0, 3))


def make_in_maps(q, k, v, wq, bq, wk, bk, wv, bv):
    scale = 1.0 / math.sqrt(DK)
    wq_s = _pack_w((np.asarray(wq, np.float32) * scale).astype(NP_BF16))
    wk_b = _pack_w(np.asarray(wk, np.float32).astype(NP_BF16))
    wv_b = _pack_w(np.asarray(wv, np.float32).astype(NP_BF16))
    bq_col = np.ascontiguousarray(
        (np.asarray(bq, np.float32) * scale).reshape(128, 1))
    ones = np.ones((128, 1), NP_BF16)
    ident = np.eye(128, dtype=NP_BF16)

    in_maps = []
    for core in range(N_CORES):
        b, h = core // 2, core % 2
        qTb = _pack_x(np.asarray(q[b], np.float32).T.astype(NP_BF16), NQB)
        kTb = _pack_x(
            np.asarray(k[b], np.float32).T[:, h * SK:(h + 1) * SK].astype(NP_BF16),
            NKB)
        vTb = _pack_x(
            np.asarray(v[b], np.float32).T[:, h * SK:(h + 1) * SK].astype(NP_BF16),
            NKB)
        in_maps.append({
            "qT": qTb, "kT": kTb, "vT": vTb,
            "wq": wq_s, "wk": wk_b, "wv": wv_b,
            "bq": bq_col, "ones": ones, "ident": ident,
        })
    return in_maps


def kernel(q, k, v, wq, bq, wk, bk, wv, bv, _trace=False, _tmpdir=None):
    nc = _build()
    in_maps = make_in_maps(q, k, v, wq, bq, wk, bk, wv, bv)
    res = run_bass_kernel_spmd(
        nc, in_maps, list(range(N_CORES)), trace=_trace, tmpdir=_tmpdir
    )
    bv_f = np.asarray(bv, np.float32)
    out = np.empty((B, S, DV), np.float32)
    for b in range(B):
        r0, r1 = res.results[2 * b], res.results[2 * b + 1]
        o = r0["outT"] + r1["outT"]
        sm = r0["sums"] + r1["sums"]
        out[b] = (o / sm).T + bv_f
    if _trace:
        kernel.last_results = res
    return out
